# revision 8
# baseline (speedup 1.0000x reference)
"""Multi-head self-attention kernel for Trainium2 (Bass/Tile), 8 NeuronCores.

Problem (hardcoded): x [4096, 512] f32; per-head Linear(512, 512) with weight
W[h] [512, 512] (torch [out, in]) and bias b[h] [512]; h = x @ W[h].T + b[h];
scores = h @ h.T; attn = softmax(scores, -1); out_h = attn @ x; final output
is the head-major concat [4096, 8*512].

Sharding: head parallel - core c computes head c entirely (its own W/b slice
arrives via the per-core input map); the host concatenates the 8 per-head
[4096, 512] outputs along the feature axis.

Numerics: all large matmuls run as fp8e4 with DoubleRow perf mode (0.5
cycles/row, the PE peak rate). Softmax bias is the Gram-matrix diagonal
s_qq (the row max here), extracted directly from the scores PSUM; softmax
is exactly invariant to a per-row bias. The attention diagonal is computed
in high precision by construction: the diag of exp(s - s_qq) is forced to
0 before exp (subtract 1e9 on the diagonal), the AV matmul accumulates
I @ x_f16 into the same PSUM group (the diag attention weight is exactly
1), and the row-sum gets +1 to account for it.
"""
import numpy as np
from contextlib import ExitStack

N, D, H = 4096, 512, 8
P = 128
NB = N // P          # 32 n-blocks
DB = D // P          # 4 d-chunks
NC4 = 4              # phase-2 column chunks of 1024
N_CORES = 8

_CACHE = {}


def _build(reps: int = 1):
    from concourse import bacc, tile, mybir, masks

    dt = mybir.dt
    f32, f32r, f16 = dt.float32, dt.float32r, dt.float16
    f8 = dt.float8e4
    DR = mybir.MatmulPerfMode.DoubleRow
    AF = mybir.ActivationFunctionType
    ALU = mybir.AluOpType

    nc = bacc.Bacc("TRN2", target_bir_lowering=False, debug=False)

    X = nc.dram_tensor("x", [N, D], f32, kind="ExternalInput")
    W = nc.dram_tensor("w", [D, D], f32, kind="ExternalInput")
    B = nc.dram_tensor("b", [D, 1], f32, kind="ExternalInput")
    OUT = nc.dram_tensor("out", [N, D], f32, kind="ExternalOutput")

    with tile.TileContext(nc) as tc, ExitStack() as ctx:
        # ---- persistent pools -------------------------------------------
        const_pool = ctx.enter_context(tc.tile_pool(name="const", bufs=1))
        x_pool = ctx.enter_context(tc.tile_pool(name="x", bufs=1))
        hT_pool = ctx.enter_context(tc.tile_pool(name="hT", bufs=1))

        ident = const_pool.tile([P, P], f32)
        masks.make_identity(nc, ident[:])
        ident_h = const_pool.tile([P, P], f16)
        nc.vector.tensor_copy(ident_h[:], ident[:])
        ident_8 = const_pool.tile([P, P], f8)
        nc.vector.tensor_copy(ident_8[:], ident[:])
        # big identity for forcing the diagonal of exp(s - d) to 0
        bigI = const_pool.tile([P, P], f32)
        nc.vector.tensor_scalar(bigI[:], ident[:], 1e9, None, op0=ALU.mult)
        b_sb = const_pool.tile([P, DB], f32)
        for ob in range(DB):
            nc.sync.dma_start(b_sb[:, ob : ob + 1], B.ap()[ob * P : (ob + 1) * P, :])

        # x in natural layout: x_sb[p, j, d] = x[j*128 + p, d]
        x_sb = x_pool.tile([P, NB, D], f16)
        x8 = x_pool.tile([P, NB, D], f8)

        # hT[p, dc, n] = h[n, dc*128 + p]
        hT = hT_pool.tile([P, DB, N], f8)

        for rep in range(reps):
            # ---- phase 1: hT = (x @ W.T + b).T ------------------------------
            with ExitStack() as p1:
                w_pool = p1.enter_context(tc.tile_pool(name=f"wp{rep}", bufs=1))
                xT_pool = p1.enter_context(tc.tile_pool(name=f"xTp{rep}", bufs=2))
                xf_pool = p1.enter_context(tc.tile_pool(name=f"xf{rep}", bufs=6))
                tr_ps_pool = p1.enter_context(
                    tc.tile_pool(name=f"p1tr{rep}", bufs=2, space="PSUM")
                )
                h_ps_pool = p1.enter_context(
                    tc.tile_pool(name=f"p1h{rep}", bufs=4, space="PSUM")
                )

                def load_x_block(j, xT, jslot):
                    xf = xf_pool.tile([P, D], f32, tag="xf")
                    nc.sync.dma_start(xf[:], X.ap()[j * P : (j + 1) * P, :])
                    nc.gpsimd.tensor_copy(x_sb[:, j, :], xf[:])
                    nc.vector.tensor_copy(x8[:, j, :], xf[:])
                    tp = tr_ps_pool.tile([P, DB, P], f16, tag="tr")
                    for dc in range(DB):
                        nc.tensor.transpose(
                            tp[:, dc, :],
                            x_sb[:, j, dc * P : (dc + 1) * P],
                            ident_h[:],
                        )
                    nc.vector.tensor_copy(
                        xT[:, :, jslot * P : (jslot + 1) * P], tp[:]
                    )

                # chunk-0 x blocks first: earliest PE work is their transposes
                xT0 = xT_pool.tile([P, DB, 512], f8, tag="xT")
                for j in range(4):
                    load_x_block(j, xT0, j)

                w_f32 = w_pool.tile([P, DB, D], f32)
                for ob in range(DB):
                    nc.sync.dma_start(
                        w_f32[:, ob, :], W.ap()[ob * P : (ob + 1) * P, :]
                    )
                w_sb = w_pool.tile([P, DB, D], f16)
                for ob in range(DB):
                    nc.vector.tensor_copy(w_sb[:, ob, :], w_f32[:, ob, :])
                # wT[p, dc, o] = W[o, dc*128 + p]
                wT = w_pool.tile([P, DB, D], f8)
                for ob in range(DB):
                    tp = tr_ps_pool.tile([P, DB, P], f16, tag="tr")
                    for dc in range(DB):
                        nc.tensor.transpose(
                            tp[:, dc, :],
                            w_sb[:, ob, dc * P : (dc + 1) * P],
                            ident_h[:],
                        )
                    nc.vector.tensor_copy(wT[:, :, ob * P : (ob + 1) * P], tp[:])

                for nc512 in range(N // 512):
                    lo, hi = nc512 * 512, (nc512 + 1) * 512
                    if nc512 == 0:
                        xT = xT0
                    else:
                        xT = xT_pool.tile([P, DB, 512], f8, tag="xT")
                        for j2 in range(4):
                            load_x_block(nc512 * 4 + j2, xT, j2)
                    for ob in range(DB):
                        hp = h_ps_pool.tile([P, 512], f32, tag="h")
                        for c in range(DB // 2):
                            nc.tensor.matmul(
                                hp[:],
                                wT[:, 2 * c : 2 * c + 2, ob * P : (ob + 1) * P],
                                xT[:, 2 * c : 2 * c + 2, :],
                                start=(c == 0),
                                stop=(c == DB // 2 - 1),
                                perf_mode=DR,
                            )
                        nc.scalar.activation(
                            hT[:, ob, lo:hi],
                            hp[:],
                            AF.Identity,
                            bias=b_sb[:, ob : ob + 1],
                            scale=1.0,
                        )

            # ---- phase 2: per q-block scores/softmax/AV ---------------------
            p2 = ctx.enter_context(ExitStack()) if reps == 1 else ExitStack()
            E_pool = p2.enter_context(tc.tile_pool(name=f"E{rep}", bufs=2))
            ET_pool = p2.enter_context(tc.tile_pool(name=f"ET{rep}", bufs=2))
            st_pool = p2.enter_context(tc.tile_pool(name=f"st{rep}", bufs=3))
            out_pool = p2.enter_context(tc.tile_pool(name=f"outp{rep}", bufs=3))
            sc_ps_pool = p2.enter_context(
                tc.tile_pool(name=f"scps{rep}", bufs=2, space="PSUM")
            )
            tr_ps_pool2 = p2.enter_context(
                tc.tile_pool(name=f"trps{rep}", bufs=2, space="PSUM")
            )
            o_ps_pool = p2.enter_context(
                tc.tile_pool(name=f"ops{rep}", bufs=2, space="PSUM")
            )

            state = {}  # per-Q tiles carried from scores-stage to drain-stage

            def scores_stage(Q):
                E_t = E_pool.tile([P, NB, P], f16, tag="E")
                acc = st_pool.tile([P, NC4], f32, tag="acc")
                bias_col = st_pool.tile([P, 1], f32, tag="bias")
                cd = Q // 8  # 1024-chunk holding the diagonal block
                for c in [cd] + [c for c in range(NC4) if c != cd]:
                    s_ps = sc_ps_pool.tile([P, 8, P], f32, tag="s")
                    for half in range(2):
                        cols = slice(c * 1024 + half * 512, c * 1024 + half * 512 + 512)
                        for k in range(DB // 2):
                            nc.tensor.matmul(
                                s_ps[:, 4 * half : 4 * half + 4, :],
                                hT[:, 2 * k : 2 * k + 2, Q * P : (Q + 1) * P],
                                hT[:, 2 * k : 2 * k + 2, cols],
                                start=(k == 0),
                                stop=(k == DB // 2 - 1),
                                perf_mode=DR,
                            )
                    if c == cd:
                        off = Q % 8
                        # bias_col = -s_qq (diag of this block); then force the
                        # diagonal of the pre-exp scores to -inf so exp -> 0
                        dtmp = st_pool.tile([P, P], f32, tag="dtmp")
                        nc.vector.scalar_tensor_tensor(
                            dtmp[:],
                            s_ps[:, off, :],
                            -1.0,
                            ident[:],
                            op0=ALU.mult,
                            op1=ALU.mult,
                        )
                        nc.vector.tensor_reduce(
                            bias_col[:], dtmp[:], axis=mybir.AxisListType.X,
                            op=ALU.add,
                        )
                        nc.vector.tensor_sub(
                            s_ps[:, off, :], s_ps[:, off, :], bigI[:]
                        )
                    nc.scalar.activation(
                        E_t[:, 8 * c : 8 * c + 8, :],
                        s_ps[:],
                        AF.Exp,
                        bias=bias_col[:, 0:1],
                        scale=1.0,
                        accum_out=acc[:, c : c + 1],
                    )
                state[Q] = (E_t, acc)

            def drain_stage(Q):
                E_t, acc = state.pop(Q)
                ET_t = ET_pool.tile([P, NB, P], f8, tag="ET")
                for g in range(8):
                    t_ps = tr_ps_pool2.tile([P, 4, P], f16, tag="t")
                    for t in range(4):
                        nc.tensor.transpose(
                            t_ps[:, t, :], E_t[:, g * 4 + t, :], ident_h[:]
                        )
                    nc.vector.tensor_copy(ET_t[:, 4 * g : 4 * g + 4, :], t_ps[:])

                rowsum = st_pool.tile([P, 1], f32, tag="rs")
                nc.vector.tensor_reduce(
                    rowsum[:], acc[:], axis=mybir.AxisListType.X, op=ALU.add
                )
                # +1 for the (excluded) unit diagonal of the attention matrix
                nc.vector.tensor_scalar(
                    rowsum[:], rowsum[:], 1.0, None, op0=ALU.add
                )
                recip = st_pool.tile([P, 1], f32, tag="rcp")
                nc.vector.reciprocal(recip[:], rowsum[:])

                o_ps = o_ps_pool.tile([P, D], f32, tag="o")
                for k in range(NB // 2):
                    nc.tensor.matmul(
                        o_ps[:],
                        ET_t[:, 2 * k : 2 * k + 2, :],
                        x8[:, 2 * k : 2 * k + 2, :],
                        start=(k == 0),
                        stop=False,
                        perf_mode=DR,
                    )
                # diag contribution at f16 precision closes the group (kept
                # last: its inputs are ready from the start, and a dep-
                # scheduled hoist of an open 'start' would wedge the bank)
                nc.tensor.matmul(
                    o_ps[:],
                    ident_h[:],
                    x_sb[:, Q, :],
                    start=False,
                    stop=True,
                )
                out_sb = out_pool.tile([P, D], f32, tag="out")
                nc.vector.tensor_scalar(
                    out_sb[:], o_ps[:], recip[:, 0:1], None, op0=ALU.mult
                )
                nc.sync.dma_start(OUT.ap()[Q * P : (Q + 1) * P, :], out_sb[:])

            # software-pipelined: emit scores(Q+1) before drain(Q) so the PE
            # has work while the Activation engine runs exp(Q)
            scores_stage(0)
            for Q in range(1, NB):
                scores_stage(Q)
                drain_stage(Q - 1)
            drain_stage(NB - 1)
            if reps != 1:
                p2.close()

    nc.compile()
    return nc


def _get_nc(reps: int = 1):
    key = ("nc", reps)
    if key not in _CACHE:
        _CACHE[key] = _build(reps)
    return _CACHE[key]


def kernel(x_resting: np.ndarray, W: np.ndarray, b: np.ndarray) -> np.ndarray:
    from concourse.bass_utils import run_bass_kernel_spmd

    nc = _get_nc()
    in_maps = [
        {
            "x": np.ascontiguousarray(x_resting, dtype=np.float32),
            "w": np.ascontiguousarray(W[c], dtype=np.float32),
            "b": np.ascontiguousarray(b[c].reshape(D, 1), dtype=np.float32),
        }
        for c in range(N_CORES)
    ]
    res = run_bass_kernel_spmd(nc, in_maps, list(range(N_CORES)))
    return np.concatenate([res.results[c]["out"] for c in range(N_CORES)], axis=1)
